# revision 6
# baseline (speedup 1.0000x reference)
"""AdaptiveSparseMoE Trainium2 kernel — 8-core SPMD.

Sharding: tokens (B*L = 16384) split 8 ways -> 2048 tokens/core (each core
holds half of one batch b = core//2). Expert-parallel: core c owns expert c.

Device pipeline per core:
  gating matmul (fp32, x^T fed from host) -> softmax/top2/dispatch ->
  AllGather of per-expert assign counts (capacity prefix across the 2 cores
  of each b) -> capacity mask via tensor_tensor_scan cumsum ->
  pooling matmul (bf16) -> AllToAll pooled partials -> expert matmul (bf16,
  W^T fed from host) -> AllToAll expert outputs -> combine matmul (bf16) ->
  fp32 output.  Aux-loss partial sums are tiny extra outputs, finished on host.

Key algebraic fact used: `modified` in the reference == base_probs exactly
(the adaptive factor is a per-token scalar that cancels in renormalization),
so top-k runs directly on softmax probs; entropy only feeds the aux loss.
"""

import os
import numpy as np
import ml_dtypes

import concourse.bass as bass
import concourse.bacc as bacc
import concourse.mybir as mybir
import concourse.tile as tile
from concourse.bass_utils import run_bass_kernel_spmd

F32 = mybir.dt.float32
BF16 = mybir.dt.bfloat16
BF = ml_dtypes.bfloat16

B, L, D, E = 4, 4096, 2048, 8
NC = 8              # cores
TPC = (B * L) // NC  # tokens per core = 2048
P = 128
NT = TPC // P       # 16 token tiles
ND = D // P         # 16 contraction chunks
CAP = 2560.0        # int(1.25 * (B*L/E) + 0.9999)
EPS = 1e-8

ts = bass.ts

_CACHE = {}
LAST_RESULTS = None


def _install_ntff_hook():
    """Register the axon NTFF profiling hook (missing antenv.axon_hooks in
    this image) so run_bass_kernel_spmd(trace=True) can report exec_time_ns."""
    import sys, types, ctypes, contextlib
    if "antenv.axon_hooks" in sys.modules:
        return
    try:
        lib = ctypes.CDLL("/opt/axon/libaxon_pjrt.so")
    except OSError:
        return
    if not hasattr(lib, "axon_start_nrt_profile"):
        return
    lib.axon_start_nrt_profile.argtypes = [ctypes.POINTER(ctypes.c_int64),
                                           ctypes.c_size_t]
    lib.axon_start_nrt_profile.restype = ctypes.c_int64
    lib.axon_stop_nrt_profile.argtypes = [ctypes.c_char_p]
    lib.axon_stop_nrt_profile.restype = ctypes.c_int64

    @contextlib.contextmanager
    def _hook(output_dir, device_ids):
        import jax
        jax.devices()
        if device_ids:
            ids = (ctypes.c_int64 * len(device_ids))(*device_ids)
            rc = lib.axon_start_nrt_profile(ids, len(device_ids))
        else:
            rc = lib.axon_start_nrt_profile(None, 0)
        if rc != 0:
            raise RuntimeError(f"axon_start_nrt_profile rc={rc}")
        try:
            yield
        finally:
            n = lib.axon_stop_nrt_profile(str(output_dir).encode())
            print(f"ntff profile: {n} file(s) -> {output_dir}")

    mod = types.ModuleType("antenv.axon_hooks")
    mod.get_axon_ntff_profile_hook = lambda: _hook
    mod.set_axon_ntff_profile_hook = lambda h: None
    import antenv
    antenv.axon_hooks = mod
    sys.modules["antenv.axon_hooks"] = mod


def _build():
    nc = bacc.Bacc("TRN2", target_bir_lowering=False, debug=False,
                   num_devices=NC)
    # ---- external I/O (per-core shards) ----
    xT = nc.dram_tensor("xT", [D, TPC], F32, kind="ExternalInput").ap()
    xb = nc.dram_tensor("xb", [TPC, D], BF16, kind="ExternalInput").ap()
    wT = nc.dram_tensor("wT", [D, D], BF16, kind="ExternalInput").ap()
    gWT = nc.dram_tensor("gWT", [D, E], F32, kind="ExternalInput").ap()
    gb = nc.dram_tensor("gb", [1, E], F32, kind="ExternalInput").ap()
    expb = nc.dram_tensor("expb", [1, D], BF16, kind="ExternalInput").ap()
    Smat = nc.dram_tensor("Smat", [P, P], F32, kind="ExternalInput").ap()
    Amat = nc.dram_tensor("Amat", [P, E], F32, kind="ExternalInput").ap()
    M1mat = nc.dram_tensor("M1mat", [E, P], F32, kind="ExternalInput").ap()
    PMmat = nc.dram_tensor("PMmat", [E, E], F32, kind="ExternalInput").ap()
    selv = nc.dram_tensor("selv", [E, 1], F32, kind="ExternalInput").ap()
    identm = nc.dram_tensor("identm", [P, P], F32, kind="ExternalInput").ap()
    out = nc.dram_tensor("out", [TPC, D], F32, kind="ExternalOutput").ap()
    stats = nc.dram_tensor("stats", [32], F32, kind="ExternalOutput").ap()

    rg = [list(range(NC))]

    with tile.TileContext(nc) as tc:
        with (
            tc.tile_pool(name="consts", bufs=1) as consts,
            tc.tile_pool(name="xtp", bufs=3) as xtp,
            tc.tile_pool(name="xbp", bufs=3) as xbp,
            tc.tile_pool(name="wtp", bufs=3) as wtp,
            tc.tile_pool(name="work", bufs=1) as work,
            tc.tile_pool(name="outp", bufs=3) as outp,
            tc.tile_pool(name="pbig", bufs=1, space="PSUM") as pbig,
            tc.tile_pool(name="ptr", bufs=1, space="PSUM") as ptr,
            tc.tile_pool(name="psmall", bufs=1, space="PSUM") as psmall,
            tc.tile_pool(name="pcomb", bufs=2, space="PSUM") as pcomb,
            tc.tile_pool(name="dram", bufs=1, space="DRAM") as dram,
        ):
            # ---------- constants ----------
            cgWT = consts.tile([P, ND * E], F32, tag="cgWT")   # [128,(16,8)]
            nc.sync.dma_start(cgWT, gWT.rearrange("(k p) e -> p k e", p=P))
            cgb = consts.tile([1, E], F32, tag="cgb")
            nc.sync.dma_start(cgb, gb)
            cexpb = consts.tile([1, D], BF16, tag="cexpb")
            nc.sync.dma_start(cexpb, expb)
            cS = consts.tile([P, P], F32, tag="cS")
            nc.sync.dma_start(cS, Smat)
            cA = consts.tile([P, E], F32, tag="cA")
            nc.sync.dma_start(cA, Amat)
            cM1 = consts.tile([E, P], F32, tag="cM1")
            nc.sync.dma_start(cM1, M1mat)
            cPM = consts.tile([E, E], F32, tag="cPM")
            nc.sync.dma_start(cPM, PMmat)
            csel = consts.tile([E, 1], F32, tag="csel")
            nc.sync.dma_start(csel, selv)
            cid = consts.tile([P, P], F32, tag="cid")
            nc.sync.dma_start(cid, identm)
            ones512 = consts.tile([1, 512], F32, tag="ones512")
            nc.vector.memset(ones512, 1.0)
            ones128 = consts.tile([P, 1], F32, tag="ones128")
            nc.vector.memset(ones128, 1.0)
            onesb4 = consts.tile([1, 4], BF16, tag="onesb4")
            nc.vector.memset(onesb4, 1.0)

            # ---------- gating: logitsT [8, 2048] = gWT.T @ xT (+ gb) ----------
            pg = pbig.tile([E, TPC], F32, tag="big")
            for k in range(ND):
                xt_k = xtp.tile([P, TPC], F32, tag="xt")
                nc.sync.dma_start(xt_k, xT[ts(k, P), :])
                for c in range(4):
                    nc.tensor.matmul(pg[:, ts(c, 512)], cgWT[:, ts(k, E)],
                                     xt_k[:, ts(c, 512)],
                                     start=(k == 0), stop=False)
            for c in range(4):
                nc.tensor.matmul(pg[:, ts(c, 512)], cgb, ones512,
                                 start=False, stop=True)
            ls = work.tile([E, TPC], F32, tag="ls")
            nc.vector.tensor_copy(ls, pg)

            # transpose to L [128 tokens, (16 tiles, 8 e)]
            ptL = ptr.tile([P, P], F32, tag="tr")
            for i in range(NT):
                nc.tensor.transpose(ptL[:, ts(i, E)], ls[:, ts(i, P)],
                                    cid[:E, :E])
            Lt = work.tile([P, P], F32, tag="Lt")
            nc.vector.tensor_copy(Lt, ptL)

            # ---------- softmax over e (grouped) ----------
            L3 = Lt.rearrange("p (i e) -> p i e", e=E)
            rmax = work.tile([P, NT], F32, tag="rmax")
            nc.vector.reduce_max(rmax, L3, axis=mybir.AxisListType.X)
            rmaxb = rmax[:, :, None].broadcast_to((P, NT, E))
            Lsub = work.tile([P, P], F32, tag="Lsub")
            nc.vector.tensor_tensor(Lsub.rearrange("p (i e) -> p i e", e=E),
                                    L3, rmaxb, op=mybir.AluOpType.subtract)
            P0 = work.tile([P, P], F32, tag="P0")
            nc.scalar.activation(P0, Lsub, mybir.ActivationFunctionType.Exp)
            rsum = work.tile([P, NT], F32, tag="rsum")
            nc.vector.reduce_sum(rsum, P0.rearrange("p (i e) -> p i e", e=E),
                                 axis=mybir.AxisListType.X)
            rinv = work.tile([P, NT], F32, tag="rinv")
            nc.vector.reciprocal(rinv, rsum)
            rinvb = rinv[:, :, None].broadcast_to((P, NT, E))
            Pt = work.tile([P, P], F32, tag="Pt")
            P3 = Pt.rearrange("p (i e) -> p i e", e=E)
            nc.vector.tensor_tensor(P3, P0.rearrange("p (i e) -> p i e", e=E),
                                    rinvb, op=mybir.AluOpType.mult)

            # ---------- top-2 dispatch ----------
            mask1 = work.tile([P, P], F32, tag="mask1")
            nc.vector.tensor_tensor(mask1.rearrange("p (i e) -> p i e", e=E),
                                    P3, rinvb, op=mybir.AluOpType.is_ge)
            T1 = work.tile([P, P], F32, tag="T1")
            nc.vector.scalar_tensor_tensor(T1, mask1, -2.0, Pt,
                                           op0=mybir.AluOpType.mult,
                                           op1=mybir.AluOpType.add)
            m2 = work.tile([P, NT], F32, tag="m2")
            nc.vector.reduce_max(m2, T1.rearrange("p (i e) -> p i e", e=E),
                                 axis=mybir.AxisListType.X)
            m2b = m2[:, :, None].broadcast_to((P, NT, E))
            wn = work.tile([P, NT], F32, tag="wn")
            nc.vector.tensor_tensor(wn, rinv, m2, op=mybir.AluOpType.add)
            nc.vector.tensor_scalar_max(wn, wn, 1e-9)
            wr = work.tile([P, NT], F32, tag="wr")
            nc.vector.reciprocal(wr, wn)
            wrb = wr[:, :, None].broadcast_to((P, NT, E))
            topm = work.tile([P, P], F32, tag="topm")
            nc.vector.tensor_tensor(topm.rearrange("p (i e) -> p i e", e=E),
                                    P3, m2b, op=mybir.AluOpType.is_ge)
            d0 = work.tile([P, P], F32, tag="d0")
            nc.vector.tensor_tensor(d0, Pt, topm, op=mybir.AluOpType.mult)
            disp = work.tile([P, P], F32, tag="disp")
            nc.vector.tensor_tensor(disp.rearrange("p (i e) -> p i e", e=E),
                                    d0.rearrange("p (i e) -> p i e", e=E),
                                    wrb, op=mybir.AluOpType.mult)
            assign = work.tile([P, P], F32, tag="assign")
            nc.vector.tensor_scalar(assign, disp, 0.0, None,
                                    op0=mybir.AluOpType.is_gt)

            # ---------- aux partials from probs ----------
            pe_g = work.tile([P, E], F32, tag="pe_g")
            nc.vector.reduce_sum(pe_g, Pt.rearrange("p (i e) -> p e i", e=E),
                                 axis=mybir.AxisListType.X)
            Peps = work.tile([P, P], F32, tag="Peps")
            nc.vector.tensor_scalar_add(Peps, Pt, EPS)
            lnP = work.tile([P, P], F32, tag="lnP")
            nc.scalar.activation(lnP, Peps, mybir.ActivationFunctionType.Ln)
            plog = work.tile([P, P], F32, tag="plog")
            nc.vector.tensor_tensor(plog, Pt, lnP, op=mybir.AluOpType.mult)
            ent1 = work.tile([P, 1], F32, tag="ent1")
            nc.vector.reduce_sum(ent1, plog, axis=mybir.AxisListType.X)
            pspg = psmall.tile([E, 1], F32, tag="sm")
            nc.tensor.matmul(pspg, pe_g, ones128, start=True, stop=True)
            pg_s = work.tile([E, 1], F32, tag="pg_s")
            nc.vector.tensor_copy(pg_s, pspg)
            psent = psmall.tile([1, 1], F32, tag="sm")
            nc.tensor.matmul(psent, ent1, ones128, start=True, stop=True)
            ent_s = work.tile([1, 1], F32, tag="ent_s")
            nc.vector.tensor_copy(ent_s, psent)

            # ---------- stacked layout [(i,e), t] via one 128x128 transpose ----
            psD = ptr.tile([P, P], F32, tag="tr")
            nc.tensor.transpose(psD, disp, cid)
            stD = work.tile([P, P], F32, tag="stD")
            nc.vector.tensor_copy(stD, psD)
            psA = ptr.tile([P, P], F32, tag="tr")
            nc.tensor.transpose(psA, assign, cid)
            stA = work.tile([P, P], F32, tag="stA")
            nc.vector.tensor_copy(stA, psA)

            # ---------- capacity: cumsum + cross-core prefix ----------
            cum = work.tile([P, P], F32, tag="cum")
            nc.vector.tensor_tensor_scan(cum, stA, stA, 0.0,
                                         op0=mybir.AluOpType.add,
                                         op1=mybir.AluOpType.bypass)
            Ttot = cum[:, P - 1:P]
            psac = psmall.tile([E, 1], F32, tag="sm")
            nc.tensor.matmul(psac, cA, Ttot, start=True, stop=True)
            ac_s = work.tile([E, 1], F32, tag="ac_s")
            nc.vector.tensor_copy(ac_s, psac)
            agin = dram.tile([E, 1], F32, tag="agin")
            nc.sync.dma_start(agin, ac_s)
            agout = dram.tile([E * NC, 1], F32, tag="agout",
                              addr_space="Shared")
            nc.gpsimd.collective_compute(
                "AllGather", mybir.AluOpType.bypass, replica_groups=rg,
                ins=[agin.opt()], outs=[agout.opt()])
            acg = work.tile([E, E], F32, tag="acg")
            nc.sync.dma_start(acg, agout.rearrange("(r e) o -> r (e o)", e=E))
            psoT = psmall.tile([E, 1], F32, tag="sm")
            nc.tensor.matmul(psoT, acg, csel, start=True, stop=True)
            offT = work.tile([E, 1], F32, tag="offT")
            nc.vector.tensor_copy(offT, psoT)
            psoff = psmall.tile([P, 1], F32, tag="sm")
            nc.tensor.matmul(psoff, cM1, offT, start=True, stop=False)
            nc.tensor.matmul(psoff, cS, Ttot, start=False, stop=True)
            offs = work.tile([P, 1], F32, tag="offs")
            nc.vector.tensor_copy(offs, psoff)
            keep = work.tile([P, P], F32, tag="keep")
            nc.vector.tensor_scalar(keep, cum, offs, CAP + 0.5,
                                    op0=mybir.AluOpType.add,
                                    op1=mybir.AluOpType.is_le)
            stDm = work.tile([P, P], F32, tag="stDm")
            nc.vector.tensor_tensor(stDm, stD, keep, op=mybir.AluOpType.mult)

            # post-capacity weighted sums (counts & util)
            rowsD = work.tile([P, 1], F32, tag="rowsD")
            nc.vector.reduce_sum(rowsD, stDm, axis=mybir.AxisListType.X)
            psws = psmall.tile([E, 1], F32, tag="sm")
            nc.tensor.matmul(psws, cA, rowsD, start=True, stop=True)
            ws_s = work.tile([E, 1], F32, tag="ws_s")
            nc.vector.tensor_copy(ws_s, psws)

            # dispatch back to [t, (i,e)] (pooling lhsT), bf16
            psR = ptr.tile([P, P], F32, tag="tr")
            nc.tensor.transpose(psR, stDm, cid)
            dispM = work.tile([P, P], BF16, tag="dispM")
            nc.vector.tensor_copy(dispM, psR)

            # dispatchT repack [(i,e),t] -> [e, 2048] via DRAM bounce, bf16
            stDmb = work.tile([P, P], BF16, tag="stDmb")
            nc.scalar.activation(stDmb, stDm,
                                 mybir.ActivationFunctionType.Copy)
            scrD = dram.tile([P, P], BF16, tag="scrD")
            nc.sync.dma_start(scrD, stDmb)
            dT8 = work.tile([E, TPC], BF16, tag="dT8")
            nc.sync.dma_start(dT8, scrD.rearrange("(i r) c -> r i c", r=E))

            # ---------- pooling: pooled[e, d] += disp^T @ x ----------
            pp = pbig.tile([E, D], F32, tag="big")
            for j in range(NT // 2):
                xbt = xbp.tile([P, 2 * D], BF16, tag="xbt")
                nc.sync.dma_start(
                    xbt, xb[2 * P * j:2 * P * (j + 1), :]
                    .rearrange("(h p) c -> p h c", p=P))
                for h in range(2):
                    i = 2 * j + h
                    for c in range(4):
                        nc.tensor.matmul(
                            pp[:, ts(c, 512)], dispM[:, ts(i, E)],
                            xbt[:, h * D + 512 * c: h * D + 512 * (c + 1)],
                            start=(i == 0), stop=(i == NT - 1))
            pps = work.tile([E, D], F32, tag="pps")
            nc.any.tensor_copy(pps, pp)

            # ---------- AllToAll #2: pooled partials + wsum ----------
            a2in = dram.tile([E, 2056], F32, tag="a2in")
            nc.sync.dma_start(a2in[:, 0:D], pps)
            nc.sync.dma_start(a2in[:, D:D + 1], ws_s)
            g2 = dram.tile([E, 2056], F32, tag="g2")
            nc.gpsimd.collective_compute(
                "AllToAll", mybir.AluOpType.bypass, replica_groups=rg,
                ins=[a2in.opt()], outs=[g2.opt()])

            Gw = work.tile([E, 1], F32, tag="Gw")
            nc.sync.dma_start(Gw, g2[:, D:D + 1])
            pspm = psmall.tile([E, 1], F32, tag="sm")
            nc.tensor.matmul(pspm, cPM, Gw, start=True, stop=True)
            cnt_s = work.tile([E, 1], F32, tag="cnt_s")
            nc.vector.tensor_scalar_max(cnt_s, pspm, 1.0)
            cri = work.tile([E, 1], F32, tag="cri")
            nc.vector.reciprocal(cri, cnt_s)
            pscri = psmall.tile([P, 1], F32, tag="sm")
            nc.tensor.matmul(pscri, cM1, cri, start=True, stop=True)
            cri128 = work.tile([P, 1], F32, tag="cri128")
            nc.vector.tensor_copy(cri128, pscri)

            R_raw = work.tile([P, P], F32, tag="R_raw")
            nc.sync.dma_start(R_raw,
                              g2[:, 0:D].rearrange("r (i c) -> i r c", c=P))
            Rs = work.tile([P, P], F32, tag="Rs")
            nc.vector.tensor_scalar(Rs, R_raw, cri128, None,
                                    op0=mybir.AluOpType.mult)
            psEI = ptr.tile([P, P], F32, tag="tr")
            nc.tensor.transpose(psEI, Rs, cid)
            eiT = work.tile([P, P], F32, tag="eiT")
            nc.vector.tensor_copy(eiT, psEI)
            eiTp = work.tile([P, P // 2], BF16, tag="eiTp")
            nc.vector.tensor_tensor(eiTp, eiT[:, 0:P:2], eiT[:, 1:P:2],
                                    op=mybir.AluOpType.add)

            # ---------- expert matmul: eo[b, f] = (ei/cnt) @ W^T + b ----------
            pe4 = pbig.tile([4, D], F32, tag="big")
            for j in range(ND // 2):
                wtt = wtp.tile([P, 2 * D], BF16, tag="wtt")
                nc.sync.dma_start(
                    wtt, wT[2 * P * j:2 * P * (j + 1), :]
                    .rearrange("(h p) c -> p h c", p=P))
                for h in range(2):
                    i = 2 * j + h
                    for c in range(4):
                        nc.tensor.matmul(
                            pe4[:, ts(c, 512)], eiTp[:, ts(i, 4)],
                            wtt[:, h * D + 512 * c: h * D + 512 * (c + 1)],
                            start=(i == 0), stop=False)
            for c in range(4):
                nc.tensor.matmul(pe4[:, ts(c, 512)], onesb4,
                                 cexpb[:, ts(c, 512)], start=False, stop=True)
            eos = work.tile([4, D], BF16, tag="eos")
            nc.any.tensor_copy(eos, pe4)

            # ---------- AllToAll #3: expert outputs ----------
            a3in = dram.tile([E, D], BF16, tag="a3in")
            a3v = a3in.rearrange("(b two) c -> b (two c)", two=2)
            nc.sync.dma_start(a3v[:, 0:D], eos)
            nc.sync.dma_start(a3v[:, D:2 * D], eos)
            g3 = dram.tile([E, D], BF16, tag="g3")
            nc.gpsimd.collective_compute(
                "AllToAll", mybir.AluOpType.bypass, replica_groups=rg,
                ins=[a3in.opt()], outs=[g3.opt()])
            eoall = work.tile([E, D], BF16, tag="eoall")
            nc.sync.dma_start(eoall, g3)

            # ---------- combine: out[t, d] = dispatchT.T @ eo ----------
            for i in range(NT):
                ot = outp.tile([P, D], F32, tag="ot")
                for c in range(4):
                    pct = pcomb.tile([P, 512], F32, tag="pct")
                    nc.tensor.matmul(pct, dT8[:, ts(i, P)],
                                     eoall[:, ts(c, 512)],
                                     start=True, stop=True)
                    nc.any.tensor_copy(ot[:, ts(c, 512)], pct)
                nc.sync.dma_start(out[ts(i, P), :], ot)

            # ---------- stats out ----------
            nc.sync.dma_start(stats[0:E], ws_s)
            nc.sync.dma_start(stats[E:2 * E], pg_s)
            nc.sync.dma_start(stats[2 * E:2 * E + 1], ent_s)

    nc.compile()
    return nc


def _consts(core):
    p = np.arange(P)
    S = ((p[:, None] % E == p[None, :] % E) &
         (p[:, None] // E < p[None, :] // E)).astype(np.float32)
    A = (p[:, None] % E == np.arange(E)[None, :]).astype(np.float32)
    PM = (np.arange(E)[:, None] // 2 ==
          np.arange(E)[None, :] // 2).astype(np.float32)
    sel = np.zeros((E, 1), np.float32)
    if core % 2 == 1:
        sel[core - 1, 0] = 1.0
    return {
        "Smat": S, "Amat": A, "M1mat": np.ascontiguousarray(A.T),
        "PMmat": PM, "selv": sel,
        "identm": np.eye(P, dtype=np.float32),
    }


def kernel(x, gate_W, gate_b, entropy_weight, confidence_weight,
           uncertainty_weight, temperature, expert_W, expert_b):
    global LAST_RESULTS
    if "nc" not in _CACHE:
        _CACHE["nc"] = _build()
    nc = _CACHE["nc"]

    x = np.asarray(x, np.float32)
    T = float(np.asarray(temperature).reshape(-1)[0])
    gWT_host = np.ascontiguousarray((np.asarray(gate_W, np.float32) / T).T)
    gb_host = (np.asarray(gate_b, np.float32) / T).reshape(1, E)
    eW = np.asarray(expert_W, np.float32)
    eb = np.asarray(expert_b, np.float32)

    in_maps = []
    for c in range(NC):
        b, half = c // 2, c % 2
        xs = x[b, half * TPC:(half + 1) * TPC, :]
        m = {
            "xT": np.ascontiguousarray(xs.T),
            "xb": xs.astype(BF),
            "wT": np.ascontiguousarray(eW[c].T).astype(BF),
            "gWT": gWT_host, "gb": gb_host,
            "expb": eb[c].reshape(1, D).astype(BF),
        }
        m.update(_consts(c))
        in_maps.append(m)

    if os.environ.get("BASS_TRACE"):
        _install_ntff_hook()
    res = run_bass_kernel_spmd(nc, in_maps, core_ids=list(range(NC)))
    LAST_RESULTS = res

    out = np.empty((B, L, D), np.float32)
    ws = np.empty((NC, E), np.float64)
    pgs = np.empty((NC, E), np.float64)
    ent = np.empty((NC,), np.float64)
    for c in range(NC):
        b, half = c // 2, c % 2
        r = res.results[c]
        out[b, half * TPC:(half + 1) * TPC, :] = r["out"]
        st = np.asarray(r["stats"], np.float64)
        ws[c] = st[0:E]
        pgs[c] = st[E:2 * E]
        ent[c] = st[2 * E]

    mean_gate = (pgs[0::2] + pgs[1::2]) / L                # (B, E)
    util = ws.sum(0) / (B * L)                             # (E,)
    mean_ent = -ent.sum() / (B * L)
    aux = (mean_gate.var() * E - util.var(ddof=1) * 0.01
           + (mean_ent - 1.0) ** 2 * 0.01)
    return out, np.float32(aux)


# revision 7
# speedup vs baseline: 1.0123x; 1.0123x over previous
"""AdaptiveSparseMoE Trainium2 kernel — 8-core SPMD.

Sharding: tokens (B*L = 16384) split 8 ways -> 2048 tokens/core (each core
holds half of one batch b = core//2). Expert-parallel: core c owns expert c.

Device pipeline per core:
  gating matmul (fp32, x^T fed from host) -> softmax/top2/dispatch ->
  pooling runs immediately on the UNMASKED dispatch while an AllGather of
  per-expert assign counts flies (capacity prefix across the 2 cores of each
  b); the capacity-dropped contribution is then subtracted by accumulating a
  NEGATED dropped-dispatch pooling pass into the same PSUM ->
  AllToAll pooled partials -> expert matmul (fp16, W^T fed from host) ->
  AllToAll expert outputs -> combine matmul (fp16) -> fp16 output, upcast on
  host.  Aux-loss partial sums are tiny extra outputs, finished on host.

Key algebraic fact used: `modified` in the reference == base_probs exactly
(the adaptive factor is a per-token scalar that cancels in renormalization),
so top-k runs directly on softmax probs; entropy only feeds the aux loss.
"""

import os
import numpy as np

import concourse.bass as bass
import concourse.bacc as bacc
import concourse.mybir as mybir
import concourse.tile as tile
from concourse.bass_utils import run_bass_kernel_spmd

F32 = mybir.dt.float32
F16 = mybir.dt.float16

B, L, D, E = 4, 4096, 2048, 8
NC = 8              # cores
TPC = (B * L) // NC  # tokens per core = 2048
P = 128
NT = TPC // P       # 16 token tiles
ND = D // P         # 16 contraction chunks
CAP = 2560.0        # int(1.25 * (B*L/E) + 0.9999)
EPS = 1e-8

ts = bass.ts

_CACHE = {}
LAST_RESULTS = None


def _install_ntff_hook():
    """Register the axon NTFF profiling hook (missing antenv.axon_hooks in
    this image) so run_bass_kernel_spmd(trace=True) can report exec_time_ns."""
    import sys, types, ctypes, contextlib
    if "antenv.axon_hooks" in sys.modules:
        return
    try:
        lib = ctypes.CDLL("/opt/axon/libaxon_pjrt.so")
    except OSError:
        return
    if not hasattr(lib, "axon_start_nrt_profile"):
        return
    lib.axon_start_nrt_profile.argtypes = [ctypes.POINTER(ctypes.c_int64),
                                           ctypes.c_size_t]
    lib.axon_start_nrt_profile.restype = ctypes.c_int64
    lib.axon_stop_nrt_profile.argtypes = [ctypes.c_char_p]
    lib.axon_stop_nrt_profile.restype = ctypes.c_int64

    @contextlib.contextmanager
    def _hook(output_dir, device_ids):
        import jax
        jax.devices()
        if device_ids:
            ids = (ctypes.c_int64 * len(device_ids))(*device_ids)
            rc = lib.axon_start_nrt_profile(ids, len(device_ids))
        else:
            rc = lib.axon_start_nrt_profile(None, 0)
        if rc != 0:
            raise RuntimeError(f"axon_start_nrt_profile rc={rc}")
        try:
            yield
        finally:
            n = lib.axon_stop_nrt_profile(str(output_dir).encode())
            print(f"ntff profile: {n} file(s) -> {output_dir}")

    mod = types.ModuleType("antenv.axon_hooks")
    mod.get_axon_ntff_profile_hook = lambda: _hook
    mod.set_axon_ntff_profile_hook = lambda h: None
    import antenv
    antenv.axon_hooks = mod
    sys.modules["antenv.axon_hooks"] = mod


def _build():
    nc = bacc.Bacc("TRN2", target_bir_lowering=False, debug=False,
                   num_devices=NC)
    # ---- external I/O (per-core shards) ----
    xT = nc.dram_tensor("xT", [D, TPC], F32, kind="ExternalInput").ap()
    xb = nc.dram_tensor("xb", [TPC, D], F16, kind="ExternalInput").ap()
    wT = nc.dram_tensor("wT", [D, D], F16, kind="ExternalInput").ap()
    gWT = nc.dram_tensor("gWT", [D, E], F32, kind="ExternalInput").ap()
    gb = nc.dram_tensor("gb", [1, E], F32, kind="ExternalInput").ap()
    expb = nc.dram_tensor("expb", [1, D], F16, kind="ExternalInput").ap()
    Smat = nc.dram_tensor("Smat", [P, P], F32, kind="ExternalInput").ap()
    Amat = nc.dram_tensor("Amat", [P, E], F32, kind="ExternalInput").ap()
    M1mat = nc.dram_tensor("M1mat", [E, P], F32, kind="ExternalInput").ap()
    PMmat = nc.dram_tensor("PMmat", [E, E], F32, kind="ExternalInput").ap()
    selv = nc.dram_tensor("selv", [E, 1], F32, kind="ExternalInput").ap()
    identm = nc.dram_tensor("identm", [P, P], F32, kind="ExternalInput").ap()
    out = nc.dram_tensor("out", [TPC, D], F16, kind="ExternalOutput").ap()
    stats = nc.dram_tensor("stats", [32], F32, kind="ExternalOutput").ap()

    rg = [list(range(NC))]

    with tile.TileContext(nc) as tc:
        with (
            tc.tile_pool(name="consts", bufs=1) as consts,
            tc.tile_pool(name="xtp", bufs=3) as xtp,
            tc.tile_pool(name="xbp", bufs=8) as xbp,
            tc.tile_pool(name="wtp", bufs=6) as wtp,
            tc.tile_pool(name="work", bufs=1) as work,
            tc.tile_pool(name="outp", bufs=3) as outp,
            tc.tile_pool(name="pbig", bufs=1, space="PSUM") as pbig,
            tc.tile_pool(name="ptr", bufs=1, space="PSUM") as ptr,
            tc.tile_pool(name="psmall", bufs=1, space="PSUM") as psmall,
            tc.tile_pool(name="pcomb", bufs=2, space="PSUM") as pcomb,
            tc.tile_pool(name="dram", bufs=1, space="DRAM") as dram,
        ):
            # ---------- constants ----------
            cgWT = consts.tile([P, ND * E], F32, tag="cgWT")   # [128,(16,8)]
            nc.sync.dma_start(cgWT, gWT.rearrange("(k p) e -> p k e", p=P))
            cgb = consts.tile([1, E], F32, tag="cgb")
            nc.sync.dma_start(cgb, gb)
            cexpb = consts.tile([1, D], F16, tag="cexpb")
            nc.sync.dma_start(cexpb, expb)
            cS = consts.tile([P, P], F32, tag="cS")
            nc.sync.dma_start(cS, Smat)
            cA = consts.tile([P, E], F32, tag="cA")
            nc.sync.dma_start(cA, Amat)
            cM1 = consts.tile([E, P], F32, tag="cM1")
            nc.sync.dma_start(cM1, M1mat)
            cPM = consts.tile([E, E], F32, tag="cPM")
            nc.sync.dma_start(cPM, PMmat)
            csel = consts.tile([E, 1], F32, tag="csel")
            nc.sync.dma_start(csel, selv)
            cid = consts.tile([P, P], F32, tag="cid")
            nc.sync.dma_start(cid, identm)
            ones512 = consts.tile([1, 512], F32, tag="ones512")
            nc.vector.memset(ones512, 1.0)
            ones128 = consts.tile([P, 1], F32, tag="ones128")
            nc.vector.memset(ones128, 1.0)
            onesb4 = consts.tile([1, 4], F16, tag="onesb4")
            nc.vector.memset(onesb4, 1.0)

            # ---------- gating: logitsT [8, 2048] = gWT.T @ xT (+ gb) --------
            pg = pbig.tile([E, TPC], F32, tag="big")
            for k in range(ND):
                xt_k = xtp.tile([P, TPC], F32, tag="xt")
                nc.sync.dma_start(xt_k, xT[ts(k, P), :])
                for c in range(4):
                    nc.tensor.matmul(pg[:, ts(c, 512)], cgWT[:, ts(k, E)],
                                     xt_k[:, ts(c, 512)],
                                     start=(k == 0), stop=False)
            for c in range(4):
                nc.tensor.matmul(pg[:, ts(c, 512)], cgb, ones512,
                                 start=False, stop=True)
            ls = work.tile([E, TPC], F32, tag="ls")
            nc.vector.tensor_copy(ls, pg)

            # x (token-major, fp16) tiles for pooling — all resident
            xbt = []
            for j in range(NT // 2):
                t = xbp.tile([P, 2 * D], F16, tag="xbt")
                nc.sync.dma_start(
                    t, xb[2 * P * j:2 * P * (j + 1), :]
                    .rearrange("(h p) c -> p h c", p=P))
                xbt.append(t)

            # transpose to L [128 tokens, (16 tiles, 8 e)]
            ptL = ptr.tile([P, P], F32, tag="tr")
            for i in range(NT):
                nc.tensor.transpose(ptL[:, ts(i, E)], ls[:, ts(i, P)],
                                    cid[:E, :E])
            Lt = work.tile([P, P], F32, tag="Lt")
            nc.vector.tensor_copy(Lt, ptL)

            # ---------- softmax over e (grouped) ----------
            L3 = Lt.rearrange("p (i e) -> p i e", e=E)
            rmax = work.tile([P, NT], F32, tag="rmax")
            nc.vector.reduce_max(rmax, L3, axis=mybir.AxisListType.X)
            rmaxb = rmax[:, :, None].broadcast_to((P, NT, E))
            Lsub = work.tile([P, P], F32, tag="Lsub")
            nc.vector.tensor_tensor(Lsub.rearrange("p (i e) -> p i e", e=E),
                                    L3, rmaxb, op=mybir.AluOpType.subtract)
            P0 = work.tile([P, P], F32, tag="P0")
            nc.scalar.activation(P0, Lsub, mybir.ActivationFunctionType.Exp)
            rsum = work.tile([P, NT], F32, tag="rsum")
            nc.vector.reduce_sum(rsum, P0.rearrange("p (i e) -> p i e", e=E),
                                 axis=mybir.AxisListType.X)
            rinv = work.tile([P, NT], F32, tag="rinv")
            nc.vector.reciprocal(rinv, rsum)
            rinvb = rinv[:, :, None].broadcast_to((P, NT, E))
            Pt = work.tile([P, P], F32, tag="Pt")
            P3 = Pt.rearrange("p (i e) -> p i e", e=E)
            nc.vector.tensor_tensor(P3, P0.rearrange("p (i e) -> p i e", e=E),
                                    rinvb, op=mybir.AluOpType.mult)

            # ---------- top-2 dispatch ----------
            mask1 = work.tile([P, P], F32, tag="mask1")
            nc.vector.tensor_tensor(mask1.rearrange("p (i e) -> p i e", e=E),
                                    P3, rinvb, op=mybir.AluOpType.is_ge)
            T1 = work.tile([P, P], F32, tag="T1")
            nc.vector.scalar_tensor_tensor(T1, mask1, -2.0, Pt,
                                           op0=mybir.AluOpType.mult,
                                           op1=mybir.AluOpType.add)
            m2 = work.tile([P, NT], F32, tag="m2")
            nc.vector.reduce_max(m2, T1.rearrange("p (i e) -> p i e", e=E),
                                 axis=mybir.AxisListType.X)
            m2b = m2[:, :, None].broadcast_to((P, NT, E))
            wn = work.tile([P, NT], F32, tag="wn")
            nc.vector.tensor_tensor(wn, rinv, m2, op=mybir.AluOpType.add)
            nc.vector.tensor_scalar_max(wn, wn, 1e-9)
            wr = work.tile([P, NT], F32, tag="wr")
            nc.vector.reciprocal(wr, wn)
            wrb = wr[:, :, None].broadcast_to((P, NT, E))
            topm = work.tile([P, P], F32, tag="topm")
            nc.vector.tensor_tensor(topm.rearrange("p (i e) -> p i e", e=E),
                                    P3, m2b, op=mybir.AluOpType.is_ge)
            d0 = work.tile([P, P], F32, tag="d0")
            nc.vector.tensor_tensor(d0, Pt, topm, op=mybir.AluOpType.mult)
            disp = work.tile([P, P], F32, tag="disp")
            nc.vector.tensor_tensor(disp.rearrange("p (i e) -> p i e", e=E),
                                    d0.rearrange("p (i e) -> p i e", e=E),
                                    wrb, op=mybir.AluOpType.mult)
            assign = work.tile([P, P], F32, tag="assign")
            nc.vector.tensor_scalar(assign, disp, 0.0, None,
                                    op0=mybir.AluOpType.is_gt)

            # ---------- pooling pass 1: ALL tokens (no capacity wait) -------
            dispA16 = work.tile([P, P], F16, tag="dispA16")
            nc.vector.tensor_copy(dispA16, disp)
            pp = pbig.tile([E, D], F32, tag="big")
            for i in range(NT):
                for c in range(4):
                    nc.tensor.matmul(
                        pp[:, ts(c, 512)], dispA16[:, ts(i, E)],
                        xbt[i // 2][:, (i % 2) * D + 512 * c:
                                    (i % 2) * D + 512 * (c + 1)],
                        start=(i == 0), stop=False)

            # ---------- aux partials from probs ----------
            pe_g = work.tile([P, E], F32, tag="pe_g")
            nc.vector.reduce_sum(pe_g, Pt.rearrange("p (i e) -> p e i", e=E),
                                 axis=mybir.AxisListType.X)
            Peps = work.tile([P, P], F32, tag="Peps")
            nc.vector.tensor_scalar_add(Peps, Pt, EPS)
            lnP = work.tile([P, P], F32, tag="lnP")
            nc.scalar.activation(lnP, Peps, mybir.ActivationFunctionType.Ln)
            plog = work.tile([P, P], F32, tag="plog")
            nc.vector.tensor_tensor(plog, Pt, lnP, op=mybir.AluOpType.mult)
            ent1 = work.tile([P, 1], F32, tag="ent1")
            nc.vector.reduce_sum(ent1, plog, axis=mybir.AxisListType.X)
            pspg = psmall.tile([E, 1], F32, tag="sm")
            nc.tensor.matmul(pspg, pe_g, ones128, start=True, stop=True)
            pg_s = work.tile([E, 1], F32, tag="pg_s")
            nc.vector.tensor_copy(pg_s, pspg)
            psent = psmall.tile([1, 1], F32, tag="sm")
            nc.tensor.matmul(psent, ent1, ones128, start=True, stop=True)
            ent_s = work.tile([1, 1], F32, tag="ent_s")
            nc.vector.tensor_copy(ent_s, psent)

            # ---------- stacked layout [(i,e), t] via one 128x128 transpose --
            psD = ptr.tile([P, P], F32, tag="tr")
            nc.tensor.transpose(psD, disp, cid)
            stD = work.tile([P, P], F32, tag="stD")
            nc.vector.tensor_copy(stD, psD)
            psA = ptr.tile([P, P], F32, tag="tr")
            nc.tensor.transpose(psA, assign, cid)
            stA = work.tile([P, P], F32, tag="stA")
            nc.vector.tensor_copy(stA, psA)

            # ---------- capacity: cumsum + cross-core prefix (AllGather) ----
            cum = work.tile([P, P], F32, tag="cum")
            nc.vector.tensor_tensor_scan(cum, stA, stA, 0.0,
                                         op0=mybir.AluOpType.add,
                                         op1=mybir.AluOpType.bypass)
            Ttot = cum[:, P - 1:P]
            psac = psmall.tile([E, 1], F32, tag="sm")
            nc.tensor.matmul(psac, cA, Ttot, start=True, stop=True)
            ac_s = work.tile([E, 1], F32, tag="ac_s")
            nc.vector.tensor_copy(ac_s, psac)
            agin = dram.tile([E, 1], F32, tag="agin")
            nc.sync.dma_start(agin, ac_s)
            agout = dram.tile([E * NC, 1], F32, tag="agout",
                              addr_space="Shared")
            nc.gpsimd.collective_compute(
                "AllGather", mybir.AluOpType.bypass, replica_groups=rg,
                ins=[agin.opt()], outs=[agout.opt()])
            acg = work.tile([E, E], F32, tag="acg")
            nc.sync.dma_start(acg, agout.rearrange("(r e) o -> r (e o)", e=E))
            psoT = psmall.tile([E, 1], F32, tag="sm")
            nc.tensor.matmul(psoT, acg, csel, start=True, stop=True)
            offT = work.tile([E, 1], F32, tag="offT")
            nc.vector.tensor_copy(offT, psoT)
            psoff = psmall.tile([P, 1], F32, tag="sm")
            nc.tensor.matmul(psoff, cM1, offT, start=True, stop=False)
            nc.tensor.matmul(psoff, cS, Ttot, start=False, stop=True)
            offs = work.tile([P, 1], F32, tag="offs")
            nc.vector.tensor_copy(offs, psoff)
            keep = work.tile([P, P], F32, tag="keep")
            nc.vector.tensor_scalar(keep, cum, offs, CAP + 0.5,
                                    op0=mybir.AluOpType.add,
                                    op1=mybir.AluOpType.is_le)
            stDm = work.tile([P, P], F32, tag="stDm")
            nc.vector.tensor_tensor(stDm, stD, keep, op=mybir.AluOpType.mult)

            # post-capacity weighted sums (counts & util)
            rowsD = work.tile([P, 1], F32, tag="rowsD")
            nc.vector.reduce_sum(rowsD, stDm, axis=mybir.AxisListType.X)
            psws = psmall.tile([E, 1], F32, tag="sm")
            nc.tensor.matmul(psws, cA, rowsD, start=True, stop=True)
            ws_s = work.tile([E, 1], F32, tag="ws_s")
            nc.vector.tensor_copy(ws_s, psws)

            # ---------- pooling pass 2: subtract capacity-dropped tokens ----
            # dropneg = (keep - 1) * stD   (negated dropped dispatch, stacked)
            dropneg = work.tile([P, P], F32, tag="dropneg")
            nc.vector.scalar_tensor_tensor(dropneg, keep, -1.0, stD,
                                           op0=mybir.AluOpType.add,
                                           op1=mybir.AluOpType.mult)
            psR = ptr.tile([P, P], F32, tag="tr")
            nc.tensor.transpose(psR, dropneg, cid)
            dispD16 = work.tile([P, P], F16, tag="dispD16")
            nc.vector.tensor_copy(dispD16, psR)
            for i in range(NT):
                for c in range(4):
                    nc.tensor.matmul(
                        pp[:, ts(c, 512)], dispD16[:, ts(i, E)],
                        xbt[i // 2][:, (i % 2) * D + 512 * c:
                                    (i % 2) * D + 512 * (c + 1)],
                        start=False, stop=(i == NT - 1))
            pps = work.tile([E, D], F32, tag="pps")
            nc.any.tensor_copy(pps, pp)

            # dispatchT repack [(i,e),t] -> [e, 2048] via DRAM bounce, fp16
            stDmb = work.tile([P, P], F16, tag="stDmb")
            nc.scalar.activation(stDmb, stDm,
                                 mybir.ActivationFunctionType.Copy)
            scrD = dram.tile([P, P], F16, tag="scrD")
            nc.sync.dma_start(scrD, stDmb)
            dT8 = work.tile([E, TPC], F16, tag="dT8")
            nc.sync.dma_start(dT8, scrD.rearrange("(i r) c -> r i c", r=E))

            # ---------- AllToAll #2: pooled partials + wsum ----------
            a2in = dram.tile([E, 2056], F32, tag="a2in")
            nc.sync.dma_start(a2in[:, 0:D], pps)
            nc.sync.dma_start(a2in[:, D:D + 1], ws_s)
            g2 = dram.tile([E, 2056], F32, tag="g2")
            nc.gpsimd.collective_compute(
                "AllToAll", mybir.AluOpType.bypass, replica_groups=rg,
                ins=[a2in.opt()], outs=[g2.opt()])

            Gw = work.tile([E, 1], F32, tag="Gw")
            nc.sync.dma_start(Gw, g2[:, D:D + 1])
            pspm = psmall.tile([E, 1], F32, tag="sm")
            nc.tensor.matmul(pspm, cPM, Gw, start=True, stop=True)
            cnt_s = work.tile([E, 1], F32, tag="cnt_s")
            nc.vector.tensor_scalar_max(cnt_s, pspm, 1.0)
            cri = work.tile([E, 1], F32, tag="cri")
            nc.vector.reciprocal(cri, cnt_s)
            pscri = psmall.tile([P, 1], F32, tag="sm")
            nc.tensor.matmul(pscri, cM1, cri, start=True, stop=True)
            cri128 = work.tile([P, 1], F32, tag="cri128")
            nc.vector.tensor_copy(cri128, pscri)

            R_raw = work.tile([P, P], F32, tag="R_raw")
            nc.sync.dma_start(R_raw,
                              g2[:, 0:D].rearrange("r (i c) -> i r c", c=P))
            Rs = work.tile([P, P], F32, tag="Rs")
            nc.vector.tensor_scalar(Rs, R_raw, cri128, None,
                                    op0=mybir.AluOpType.mult)
            psEI = ptr.tile([P, P], F32, tag="tr")
            nc.tensor.transpose(psEI, Rs, cid)
            eiT = work.tile([P, P], F32, tag="eiT")
            nc.vector.tensor_copy(eiT, psEI)
            eiTp = work.tile([P, P // 2], F16, tag="eiTp")
            nc.vector.tensor_tensor(eiTp, eiT[:, 0:P:2], eiT[:, 1:P:2],
                                    op=mybir.AluOpType.add)

            # ---------- expert matmul: eo[b, f] = (ei/cnt) @ W^T + b --------
            pe4 = pbig.tile([4, D], F32, tag="big")
            for j in range(ND // 2):
                wtt = wtp.tile([P, 2 * D], F16, tag="wtt")
                nc.sync.dma_start(
                    wtt, wT[2 * P * j:2 * P * (j + 1), :]
                    .rearrange("(h p) c -> p h c", p=P))
                for h in range(2):
                    i = 2 * j + h
                    for c in range(4):
                        nc.tensor.matmul(
                            pe4[:, ts(c, 512)], eiTp[:, ts(i, 4)],
                            wtt[:, h * D + 512 * c: h * D + 512 * (c + 1)],
                            start=(i == 0), stop=False)
            for c in range(4):
                nc.tensor.matmul(pe4[:, ts(c, 512)], onesb4,
                                 cexpb[:, ts(c, 512)], start=False, stop=True)
            eos = work.tile([4, D], F16, tag="eos")
            nc.any.tensor_copy(eos, pe4)

            # ---------- AllToAll #3: expert outputs ----------
            a3in = dram.tile([E, D], F16, tag="a3in")
            a3v = a3in.rearrange("(b two) c -> b (two c)", two=2)
            nc.sync.dma_start(a3v[:, 0:D], eos)
            nc.sync.dma_start(a3v[:, D:2 * D], eos)
            g3 = dram.tile([E, D], F16, tag="g3")
            nc.gpsimd.collective_compute(
                "AllToAll", mybir.AluOpType.bypass, replica_groups=rg,
                ins=[a3in.opt()], outs=[g3.opt()])
            eoall = work.tile([E, D], F16, tag="eoall")
            nc.sync.dma_start(eoall, g3)

            # ---------- combine: out[t, d] = dispatchT.T @ eo ----------
            for i in range(NT):
                ot = outp.tile([P, D], F16, tag="ot")
                for c in range(4):
                    pct = pcomb.tile([P, 512], F32, tag="pct")
                    nc.tensor.matmul(pct, dT8[:, ts(i, P)],
                                     eoall[:, ts(c, 512)],
                                     start=True, stop=True)
                    if c % 2 == 0:
                        nc.vector.tensor_copy(ot[:, ts(c, 512)], pct)
                    else:
                        nc.scalar.activation(
                            ot[:, ts(c, 512)], pct,
                            mybir.ActivationFunctionType.Copy)
                nc.sync.dma_start(out[ts(i, P), :], ot)

            # ---------- stats out ----------
            nc.sync.dma_start(stats[0:E], ws_s)
            nc.sync.dma_start(stats[E:2 * E], pg_s)
            nc.sync.dma_start(stats[2 * E:2 * E + 1], ent_s)

    nc.compile()
    return nc


def _consts(core):
    p = np.arange(P)
    S = ((p[:, None] % E == p[None, :] % E) &
         (p[:, None] // E < p[None, :] // E)).astype(np.float32)
    A = (p[:, None] % E == np.arange(E)[None, :]).astype(np.float32)
    PM = (np.arange(E)[:, None] // 2 ==
          np.arange(E)[None, :] // 2).astype(np.float32)
    sel = np.zeros((E, 1), np.float32)
    if core % 2 == 1:
        sel[core - 1, 0] = 1.0
    return {
        "Smat": S, "Amat": A, "M1mat": np.ascontiguousarray(A.T),
        "PMmat": PM, "selv": sel,
        "identm": np.eye(P, dtype=np.float32),
    }


def kernel(x, gate_W, gate_b, entropy_weight, confidence_weight,
           uncertainty_weight, temperature, expert_W, expert_b):
    global LAST_RESULTS
    if "nc" not in _CACHE:
        _CACHE["nc"] = _build()
    nc = _CACHE["nc"]

    x = np.asarray(x, np.float32)
    T = float(np.asarray(temperature).reshape(-1)[0])
    gWT_host = np.ascontiguousarray((np.asarray(gate_W, np.float32) / T).T)
    gb_host = (np.asarray(gate_b, np.float32) / T).reshape(1, E)
    eW = np.asarray(expert_W, np.float32)
    eb = np.asarray(expert_b, np.float32)

    in_maps = []
    for c in range(NC):
        b, half = c // 2, c % 2
        xs = x[b, half * TPC:(half + 1) * TPC, :]
        m = {
            "xT": np.ascontiguousarray(xs.T),
            "xb": xs.astype(np.float16),
            "wT": np.ascontiguousarray(eW[c].T).astype(np.float16),
            "gWT": gWT_host, "gb": gb_host,
            "expb": eb[c].reshape(1, D).astype(np.float16),
        }
        m.update(_consts(c))
        in_maps.append(m)

    if os.environ.get("BASS_TRACE"):
        _install_ntff_hook()
    res = run_bass_kernel_spmd(nc, in_maps, core_ids=list(range(NC)))
    LAST_RESULTS = res

    out = np.empty((B, L, D), np.float32)
    ws = np.empty((NC, E), np.float64)
    pgs = np.empty((NC, E), np.float64)
    ent = np.empty((NC,), np.float64)
    for c in range(NC):
        b, half = c // 2, c % 2
        r = res.results[c]
        out[b, half * TPC:(half + 1) * TPC, :] = r["out"].astype(np.float32)
        st = np.asarray(r["stats"], np.float64)
        ws[c] = st[0:E]
        pgs[c] = st[E:2 * E]
        ent[c] = st[2 * E]

    mean_gate = (pgs[0::2] + pgs[1::2]) / L                # (B, E)
    util = ws.sum(0) / (B * L)                             # (E,)
    mean_ent = -ent.sum() / (B * L)
    aux = (mean_gate.var() * E - util.var(ddof=1) * 0.01
           + (mean_ent - 1.0) ** 2 * 0.01)
    return out, np.float32(aux)
